# revision 1
# baseline (speedup 1.0000x reference)
"""Trainium2 Bass kernel for nn_FSE_Module_79147657331158.

Pipeline (per batch image, one per NeuronCore, 8-way data parallel):
  h1 = mish(BN1(conv3x3(x, w1)))          64 -> 128 ch
  h2 = mish(BN2(conv3x3(h1, w2))) + x     128 -> 64 ch
  cA, (cH,cV,cD) = haar_dwt2(h2)
  x_low  = cA
  x_high = mish(BNh(conv1x1(concat(cH,cV,cD), wh)))

Implementation notes:
  - convs are 9-tap (3x3) / 4-tap (2x2-stride-2, DWT-fused 1x1) matmul
    accumulations in PSUM, float32r (TF32-like, full PE rate at N>=256).
  - BN scale is folded into the weights host-side; BN bias is applied by
    the ACT engine during PSUM evacuation (Identity + per-partition bias).
  - mish(z) = z * (g-1)/(g+1) with g = (1+exp(z))^2: the PSUM
    evacuation is fused into the Exp (u = exp(psum + bias)); square and
    the +1 offsets run on the scalar engine (one table set, no switches),
    the reciprocal via the fast custom DVE op, and the final
    (psum+bias)*q via scalar_tensor_tensor on the vector engine.
    GPSIMD is never used (its per-op dispatch cost is ~10x the model).
  - The DWT + 1x1 conv are fused: x_high = mish(BNh(conv2x2s2(h2, W')))
    where W' combines wh with the Haar signs; x_low is computed with
    vector adds directly from h2.
  - h2 is stored column-deinterleaved ([even cols | odd cols] per row)
    and row-packed across partition halves so the 2x2-stride-2 conv taps
    and the DWT adds read contiguous spans at full 128-partition width.
"""
import os
import sys
from contextlib import ExitStack

sys.path.insert(0, "/opt/trn_rl_repo")

import numpy as np

_CACHE = {}


def _fold_params(w1, b1, g1, be1, m1, v1, w2, b2, g2, be2, m2, v2,
                 wh, bh, gh, beh, mh, vh):
    eps = 1e-5
    f64 = np.float64
    s1 = (g1.astype(f64) / np.sqrt(v1.astype(f64) + eps))
    bv1 = ((b1.astype(f64) - m1) * s1 + be1)
    w1t = (w1.astype(f64) * s1[:, None, None, None]).transpose(2, 3, 1, 0)
    w1t = np.ascontiguousarray(w1t.reshape(9, 64, 128), dtype=np.float32)

    s2 = (g2.astype(f64) / np.sqrt(v2.astype(f64) + eps))
    bv2 = ((b2.astype(f64) - m2) * s2 + be2)
    w2t = (w2.astype(f64) * s2[:, None, None, None]).transpose(2, 3, 1, 0)
    w2t = np.ascontiguousarray(w2t.reshape(9, 128, 64), dtype=np.float32)

    sh = (gh.astype(f64) / np.sqrt(vh.astype(f64) + eps))
    bvh = ((bh.astype(f64) - mh) * sh + beh)
    whm = wh[:, :, 0, 0].astype(f64)  # [64, 192]
    wH, wV, wD = whm[:, :64], whm[:, 64:128], whm[:, 128:]
    wpt = np.zeros((5, 128, 64), dtype=np.float32)
    wpt[4, :64, :] = 0.5 * np.eye(64, dtype=np.float32)
    wpt[4, 64:, :] = 0.5 * np.eye(64, dtype=np.float32)
    for a in (0, 1):
        for b in (0, 1):
            sH = 1.0 if a == 0 else -1.0
            sV = 1.0 if b == 0 else -1.0
            sD = 1.0 if a == b else -1.0
            wp = 0.5 * (wH * sH + wV * sV + wD * sD) * sh[:, None]  # [o, c]
            wpt[2 * a + b, :64, :] = wp.T.astype(np.float32)
            wpt[2 * a + b, 64:, :] = wp.T.astype(np.float32)

    bv1 = bv1.astype(np.float32).reshape(128, 1)
    bv2d = np.tile(bv2.astype(np.float32), 2).reshape(128, 1)
    bvhd = np.tile(bvh.astype(np.float32), 2).reshape(128, 1)
    return w1t, bv1, w2t, bv2d, wpt, bvhd


class _Builder:
    def __init__(self, H, W, finalize=True, reps=1, parts=None):
        self.finalize = finalize
        self.reps = reps
        self.parts = parts or {"conv1", "mish1", "conv2", "mish2",
                               "dwt", "convh"}
        import concourse.bass as bass
        import concourse.bacc as bacc
        import concourse.mybir as mybir
        from concourse.dt import dt
        from concourse.tile import TileContext
        from concourse.alu_op_type import AluOpType

        self.bass = bass
        self.bacc = bacc
        self.mybir = mybir
        self.F32, self.F32R = dt.float32, dt.float32r
        self.Act = mybir.ActivationFunctionType
        self.Alu = AluOpType
        self.H, self.W = H, W
        self.BLOCK = 16
        self.NB = H // self.BLOCK
        self.TileContext = TileContext

    def build(self):
        H, W = self.H, self.W
        F32, F32R = self.F32, self.F32R
        HW2 = (H // 2) * (W // 2)
        nc = self.bacc.Bacc(None, target_bir_lowering=False)
        self.nc = nc

        self.params = {}
        for nm, shp, dtp in (
            ("w1t", [9, 64, 128], F32R), ("w2t", [9, 128, 64], F32R),
            ("wpt", [5, 128, 64], F32R), ("bv1", [128, 1], F32),
            ("bv2", [128, 1], F32), ("bvh", [128, 1], F32),
        ):
            self.params[nm] = nc.declare_dram_parameter(nm, shp, dtp,
                                                        isOutput=False)
        # x arrives host-padded: [64, H+2 rows, W+2 cols], zero borders
        # (one col each side, two extra zero rows at the bottom)
        self.x = nc.declare_dram_parameter("x", [64, (H + 2) * (W + 2)], F32R,
                                           isOutput=False)
        xlo = nc.declare_dram_parameter("x_low", [64, HW2], F32, isOutput=True)
        xhi = nc.declare_dram_parameter("x_high", [64, HW2], F32,
                                        isOutput=True)
        self.xlo3 = xlo.rearrange("c (i j) -> c i j", j=W // 2)
        self.xhi3 = xhi.rearrange("c (i j) -> c i j", j=W // 2)

        with self.TileContext(nc) as tc:
            with ExitStack() as st:
                p = {}
                for name, bufs, space in (
                    ("const", 1, "SBUF"), ("xt", 2, "SBUF"),
                    ("h1", 2, "SBUF"), ("u", 5, "SBUF"),
                    ("g", 5, "SBUF"), ("r", 5, "SBUF"),
                    ("q", 5, "SBUF"), ("m", 3, "SBUF"), ("h2d", 2, "SBUF"),
                    ("cA", 2, "SBUF"), ("xh", 2, "SBUF"),
                    ("ps1", 3, "PSUM"), ("ps2", 3, "PSUM"),
                    ("psh", 2, "PSUM"),
                ):
                    p[name] = st.enter_context(
                        tc.tile_pool(name=name, bufs=bufs, space=space))
                self.p = p
                self._emit_constants()
                if self.reps == 1:
                    for b in range(self.NB):
                        self._emit_block(b)
                else:
                    with tc.For_i(0, self.reps, 1):
                        for b in range(self.NB):
                            self._emit_block(b)
        if self.finalize:
            nc.finalize()
        return nc

    def _dram(self, name):
        return self.params[name]

    def _emit_constants(self):
        nc, p = self.nc, self.p
        F32, F32R = self.F32, self.F32R
        self.w1s = p["const"].tile([64, 9 * 128], F32R, tag="w1s")
        nc.sync.dma_start(
            out=self.w1s.rearrange("k (t m) -> k t m", m=128),
            in_=self._dram("w1t").rearrange("t k m -> k t m"))
        self.w2s = p["const"].tile([128, 9 * 64], F32R, tag="w2s")
        nc.sync.dma_start(
            out=self.w2s.rearrange("k (t m) -> k t m", m=64),
            in_=self._dram("w2t").rearrange("t k m -> k t m"))
        self.wps = p["const"].tile([128, 5 * 64], F32R, tag="wps")
        nc.sync.dma_start(
            out=self.wps.rearrange("k (t m) -> k t m", m=64),
            in_=self._dram("wpt").rearrange("t k m -> k t m"))
        self.bv1s = p["const"].tile([128, 1], F32, tag="bv1s")
        nc.sync.dma_start(out=self.bv1s[:], in_=self._dram("bv1")[:])
        self.bv2s = p["const"].tile([128, 1], F32, tag="bv2s")
        nc.sync.dma_start(out=self.bv2s[:], in_=self._dram("bv2")[:])
        self.bvhs = p["const"].tile([128, 1], F32, tag="bvhs")
        nc.sync.dma_start(out=self.bvhs[:], in_=self._dram("bvh")[:])

    def _mish_q_from_u(self, u, cols, part=128):
        """q = (g-1)/(g+1), g = (1+u)^2, u = exp(z) precomputed.

        den overwrites u (dead after g); never touches gpsimd."""
        nc, p, W = self.nc, self.p, self.W
        F32, Act, Alu = self.F32, self.Act, self.Alu
        g = p["g"].tile([128, 2 * W], F32, tag="g")
        nc.scalar.activation(g[0:part, :cols], u[0:part, :cols], Act.Square,
                             bias=1.0)
        nc.scalar.activation(u[0:part, :cols], g[0:part, :cols], Act.Identity,
                             bias=1.0)
        r = p["r"].tile([128, 2 * W], F32, tag="r")
        nc.vector.reciprocal_approx_fast(r[0:part, :cols], u[0:part, :cols])
        q = p["q"].tile([128, 2 * W], F32, tag="q")
        nc.vector.scalar_tensor_tensor(
            q[0:part, :cols], g[0:part, :cols], -1.0, r[0:part, :cols],
            Alu.add, Alu.mult)
        return q

    def _emit_mms(self, mms):
        for i, (o, l, rr) in enumerate(mms):
            self.nc.tensor.matmul(o, l, rr, start=(i == 0),
                                  stop=(i == len(mms) - 1))

    def _conv1_group(self, a, n, psum, xtv, rx0):
        # xtv is the zero-bordered [64|128, rows, W+2] view of x rows
        H, W = self.H, self.W
        pv = psum.rearrange("p (rr c) -> p rr c", c=W)
        mms = []
        for dy in (0, -1, 1):
            for dx in (0, 1, -1):
                rows = [rr for rr in range(a, a + n) if 0 <= rr + dy <= H - 1]
                if not rows:
                    continue
                t = (dy + 1) * 3 + (dx + 1)
                i0, nr = rows[0] - a, len(rows)
                psl = pv[:, i0:i0 + nr, :]
                rsl = xtv[0:64, rows[0] + dy - rx0:rows[0] + dy - rx0 + nr,
                          dx + 1:dx + 1 + W]
                mms.append((psl, self.w1s[:, t * 128:(t + 1) * 128], rsl))
        self._emit_mms(mms)

    def _conv2_group(self, ya, psum, h1v, a0):
        H, W = self.H, self.W
        pv = psum.rearrange("p (rr c) -> p rr c", c=W)
        mms = []
        for dy in (0, -1, 1):
            for dx in (0, 1, -1):
                rows = [rr for rr in (ya, ya + 1) if 0 <= rr + dy <= H - 1]
                if not rows:
                    continue
                t = (dy + 1) * 3 + (dx + 1)
                i0, nr = rows[0] - ya, len(rows)
                psl = pv[:, i0:i0 + nr, :]
                rsl = h1v[:, rows[0] + dy - a0:rows[0] + dy - a0 + nr,
                          dx + 1:dx + 1 + W]
                mms.append((psl, self.w2s[:, t * 64:(t + 1) * 64], rsl))
        self._emit_mms(mms)

    def _emit_block(self, b):
        nc, p = self.nc, self.p
        H, W, BLOCK, NB = self.H, self.W, self.BLOCK, self.NB
        F32, F32R, Act, Alu = self.F32, self.F32R, self.Act, self.Alu
        Wh = W // 2
        r0 = b * BLOCK
        a0 = 0 if b == 0 else r0 - 1
        a1 = min(r0 + BLOCK, H - 1)
        groups = []
        a = a0
        while a <= a1:
            n = 2 if a + 1 <= a1 else 1
            groups.append((a, n))
            a += n
        rx0 = max(a0 - 1, 0)
        rx1 = min(a1 + 1, H - 1)
        nxr = rx1 - rx0 + 1

        Wp = W + 2  # zero border column on each side of every row
        xt = p["xt"].tile([64, 20 * Wp], F32R, tag="xt")
        xtv = xt.rearrange("p (rr c) -> p rr c", c=Wp)
        nc.sync.dma_start(
            out=xt[0:64, 0:nxr * Wp],
            in_=self.x[:, rx0 * Wp:(rx1 + 1) * Wp])
        xtvf = xt.bitcast(F32).rearrange("p (rr c) -> p rr c", c=Wp)

        # ---- conv1 -> h1 ----
        h1 = p["h1"].tile([128, 18 * Wp], F32R, tag="h1")
        h1v = h1.rearrange("p (rr c) -> p rr c", c=Wp)
        # zero border columns (on DVE; gpsimd ops have huge dispatch cost)
        h1vf = h1.bitcast(F32).rearrange("p (rr c) -> p rr c", c=Wp)
        nc.vector.memset(h1vf[:, 0:18, 0:1], 0.0)
        nc.vector.memset(h1vf[:, 0:18, W + 1:W + 2], 0.0)
        for (ga, gn) in groups:
            cols = gn * W
            psum = p["ps1"].tile([128, 2 * W], F32, tag="ps1")
            self._conv1_group(ga, gn, psum, xtv, rx0)
            u = p["u"].tile([128, 2 * W], F32, tag="u")
            # fused evacuation: u = exp(psum + bias)
            nc.scalar.activation(u[:, :cols], psum[:, 0:cols],
                                 Act.Exp, bias=self.bv1s[:])
            if "mish1" not in self.parts:
                continue
            q = self._mish_q_from_u(u, cols)
            lr0 = ga - a0
            nc.vector.scalar_tensor_tensor(
                h1v[:, lr0:lr0 + gn, 1:W + 1], psum[:, 0:cols],
                self.bv1s[:], q[:, :cols], Alu.add, Alu.mult)
        if "conv2" not in self.parts:
            return

        # ---- conv2 + bias + mish + residual -> h2d (row-deinterleaved) ----
        # h2d: 16 rows at partitions 0-63, each row stored [evens | odds]
        h2d = p["h2d"].tile([64, BLOCK * W], F32R, tag="h2d")
        for gi in range(8):
            ya = r0 + gi * 2
            psum = p["ps2"].tile([64, 2 * W], F32, tag="ps2")
            self._conv2_group(ya, psum, h1v, a0)
            u2 = p["u"].tile([128, 2 * W], F32, tag="u")
            nc.scalar.activation(u2[0:64, :2 * W], psum[:, 0:2 * W],
                                 Act.Exp, bias=self.bv2s[0:64])
            if "mish2" not in self.parts:
                continue
            q2 = self._mish_q_from_u(u2, 2 * W, part=64)
            mm = p["m"].tile([64, 2 * W], F32, tag="m")
            nc.vector.scalar_tensor_tensor(
                mm[:], psum[:, 0:2 * W], self.bv2s[0:64],
                q2[0:64, :2 * W], Alu.add, Alu.mult)
            dout = h2d[:, gi * 2 * W:(gi + 1) * 2 * W].rearrange(
                "p (rr pp j) -> p rr j pp", rr=2, pp=2, j=Wh)
            nc.vector.tensor_add(
                out=dout, in0=mm[:],
                in1=xtvf[0:64, ya - rx0:ya - rx0 + 2, 1:W + 1])

        if "mish2" not in self.parts:
            return
        # ---- DWT low band ----
        # pair i: rows (2i, 2i+1); A/B = row 2i evens/odds, C/D = row 2i+1
        if "dwt" not in self.parts:
            return
        # DWT low band on the PE (2x2-s2 conv, 0.5*I weights) merged
        # with convh; psum group hg covers pairs (2hg, 2hg+1)
        NP = BLOCK // 2  # pairs per block
        h2r = h2d.rearrange("p (pr two bb j) -> p pr two bb j",
                            two=2, bb=2, j=Wh)
        cat = p["cA"].tile([64, NP * Wh], F32, tag="cA")
        xht = p["xh"].tile([64, NP * Wh], F32, tag="xh")
        for hg in range(NP // 2):
            psA = p["psh"].tile([64, W], F32, tag="psh")
            mmsA = []
            for t4 in range(4):
                aa, bb = t4 // 2, t4 % 2
                rsl = h2r[:, 2 * hg:2 * hg + 2, aa, bb, :]
                mmsA.append((psA[:, :],
                             self.wps[0:64, 4 * 64:5 * 64], rsl))
            self._emit_mms(mmsA)
            nc.scalar.activation(cat[:, hg * W:(hg + 1) * W], psA[:],
                                 Act.Identity)
            psum = p["psh"].tile([64, W], F32, tag="psh")
            mms = []
            for t4 in range(4):
                aa, bb = t4 // 2, t4 % 2
                rsl = h2r[:, 2 * hg:2 * hg + 2, aa, bb, :]
                mms.append((psum[:, :],
                            self.wps[0:64, t4 * 64:(t4 + 1) * 64], rsl))
            self._emit_mms(mms)
            uh = p["u"].tile([128, 2 * W], F32, tag="u")
            nc.scalar.activation(uh[0:64, :W], psum[:],
                                 Act.Exp, bias=self.bvhs[0:64])
            qh = self._mish_q_from_u(uh, W, part=64)
            nc.vector.scalar_tensor_tensor(
                xht[:, hg * W:(hg + 1) * W], psum[:], self.bvhs[0:64],
                qh[0:64, :W], Alu.add, Alu.mult)
        nc.sync.dma_start(
            out=self.xlo3[:, NP * b:NP * (b + 1), :],
            in_=cat.rearrange("c (pr j) -> c pr j", j=Wh))
        nc.sync.dma_start(
            out=self.xhi3[:, NP * b:NP * (b + 1), :],
            in_=xht.rearrange("c (pr j) -> c pr j", j=Wh))


def _build(H, W, finalize=True, reps=1, parts=None):
    return _Builder(H, W, finalize=finalize, reps=reps, parts=parts).build()


def _get_program(H, W):
    key = (H, W)
    if key not in _CACHE:
        _CACHE[key] = _build(H, W)
    return _CACHE[key]


def kernel(x, w1, b1, g1, be1, m1, v1, w2, b2, g2, be2, m2, v2,
           wh, bh, gh, beh, mh, vh):
    from concourse.bass_utils import run_bass_kernel_spmd

    x = np.asarray(x, dtype=np.float32)
    B, C, H, W = x.shape
    w1t, bv1, w2t, bv2d, wpt, bvhd = _fold_params(
        np.asarray(w1, np.float32), np.asarray(b1, np.float32),
        np.asarray(g1, np.float32), np.asarray(be1, np.float32),
        np.asarray(m1, np.float32), np.asarray(v1, np.float32),
        np.asarray(w2, np.float32), np.asarray(b2, np.float32),
        np.asarray(g2, np.float32), np.asarray(be2, np.float32),
        np.asarray(m2, np.float32), np.asarray(v2, np.float32),
        np.asarray(wh, np.float32), np.asarray(bh, np.float32),
        np.asarray(gh, np.float32), np.asarray(beh, np.float32),
        np.asarray(mh, np.float32), np.asarray(vh, np.float32))

    nc = _get_program(H, W)
    core_ids = list(range(B))
    xp = np.zeros((B, C, H + 2, W + 2), dtype=np.float32)
    xp[:, :, 0:H, 1:W + 1] = x
    in_maps = []
    for i in range(B):
        in_maps.append({
            "x": np.ascontiguousarray(xp[i].reshape(C, (H + 2) * (W + 2))),
            "w1t": w1t, "w2t": w2t, "wpt": wpt,
            "bv1": bv1, "bv2": bv2d, "bvh": bvhd,
        })
    trace = os.environ.get("KERNEL_TRACE", "0") == "1"
    try:
        res = run_bass_kernel_spmd(nc, in_maps, core_ids, trace=trace)
    except ModuleNotFoundError:
        # NTFF trace hook unavailable in this container
        res = run_bass_kernel_spmd(nc, in_maps, core_ids, trace=False)
    if res.exec_time_ns is not None:
        print(f"HW exec time: {res.exec_time_ns} ns")
    H2, W2 = H // 2, W // 2
    x_low = np.stack([res.results[i]["x_low"].reshape(C, H2, W2)
                      for i in range(B)])
    x_high = np.stack([res.results[i]["x_high"].reshape(C, H2, W2)
                       for i in range(B)])
    return (x_low, x_high)



# revision 42
# speedup vs baseline: 2.1061x; 2.1061x over previous
"""Trainium2 Bass kernel for nn_FSE_Module_79147657331158.

Pipeline (one image per NeuronCore, 8-way batch data parallel):
  h1 = mish(BN1(conv3x3(x, w1)))          64 -> 128 ch
  h2 = mish(BN2(conv3x3(h1, w2))) + x     128 -> 64 ch
  x_low  = cA(haar_dwt2(h2))
  x_high = mish(BNh(conv1x1(concat(cH,cV,cD), wh)))

~423 us / batch on 8 cores (baseline 891 us; 2.1x).  Key ideas:
  - mish in 1 ACT + 2 custom-DVE ops per element: u = exp(z+b) [ACT],
    MISH_D computes y ~= 1/(u(u+2)+2) in one DVE instruction (bitwise-NOT
    reciprocal seed + 1 Newton step, 8 ALU stages), MISH_OUT computes
    (z+b)*(1-2y) streaming z straight from PSUM with the per-partition
    bias in a scalar slot (mish(z) = z*(g-1)/(g+1), g = (1+e^z)^2, and
    (g-1)/(g+1) = 1-2y).  vs 3 ACT + 3 DVE ops for the unfused chain.
  - conv1 in 5 matmuls per 2-row group (vs 9): dy-taps K=128-packed on
    [x ; x+1row] and [x ; x+1col] dual tiles (x, w1 in bf16).
  - conv2: two output rows packed in the PE's M dim (out chans 64).
    All taps stream FULL h1 rows (F=258) and land at per-dx column
    offsets in a [128, 260] psum window; psum itself does the dx
    re-alignment (fp32r psum dst must be 2-elem aligned, so the dx=0
    tap reads rows from col 1).  12 matmuls x F=258 per row pair =
    1.49x fewer PE cycles than 9 x F=512, and the mish runs on 128
    partitions.
  - residual on the otherwise-idle GPSIMD engine as a plain tensor_add
    (h2' = 0.5*mish2 + 0.5*x; x pre-scaled 0.5 on host, w1 x2, conv2
    out op emits 0.5*mish via a (z+b)*(0.5-y) variant).
  - DWT: h2' holds even rows on partitions 0-63, odd on 64-127, so the
    row-parity taps K-pack to K=128, and cA + x_high share each matmul's
    M halves: 2 matmuls of F=256 per supergroup total.
  - h1 is a sliding window: blocks read boundary rows from the previous
    block's tile (pool bufs=2), no halo recompute.
  - custom DVE ops NEVER run at partition offset 64 (silently broken
    there); ACT/stt are fine at offset 64.
"""
import os
import sys
from contextlib import ExitStack

sys.path.insert(0, "/opt/trn_rl_repo")

import numpy as np

_CACHE = {}

MQ_C0 = -0.23549792
MQ_C1 = 2.0017324
_MISH_OPS = None


def _register_op(name, spec):
    import concourse.dve_ops as dve_ops
    from concourse.dve_spec import lower, _has_src1
    from concourse.dve_uop import DveOpSpec

    for op in dve_ops.OPS:
        if op.name == name:
            return op
    row = max(dve_ops._SUB_OPCODE_FOR_NAME.values()) + 1
    assert row < 0x20
    dve_ops._SUB_OPCODE_FOR_NAME[name] = row
    shas = {}
    for ver in ("v3", "v4"):
        s = DveOpSpec(name=name, opcode=row, uops=lower(spec, ver=ver),
                      rd1_en=_has_src1(spec))
        shas[ver] = s.sha(ver)
    op = dve_ops.DveOp(name, spec, subdim=False, uops_sha=shas)
    dve_ops.OPS.append(op)
    dve_ops.CUSTOM_DVE_SPECS[name] = spec
    return op


def _register_mish_ops():
    """MISH_D: y ~= 1/(u(u+2)+2) (NOT-seed + 1 Newton step, 8 stages).
    MISH_OUT: out = (z + bias)*(1 - 2y) with z streamed from PSUM and the
    per-partition bias riding the s0 scalar slot (4 stages).
    mish(z) = z*(g-1)/(g+1), g=(1+e^z)^2 = u(u+2)+1, so (g-1)/(g+1)=1-2y.
    """
    global _MISH_OPS
    if _MISH_OPS is not None:
        return _MISH_OPS
    from concourse.dve_spec import Spec, Src0, Src1, C0, C1, C2, One, AluOp, Bin

    n = (Src0 + C2) * Src0
    d = n + C2
    nd = Bin(AluOp.BITWISE_NOT, d, d)
    y0 = nd * C0
    body_d = y0 * (C1 - d * y0)

    def ref_d(in0, in1, s0, s1, imm2):
        u = np.ascontiguousarray(in0.astype(np.float32))
        nn = ((u + np.float32(imm2)) * u).astype(np.float32)
        dd = (nn + np.float32(imm2)).astype(np.float32)
        ndv = (~dd.view(np.int32)).view(np.float32)
        yy0 = (ndv * np.float32(s0)).astype(np.float32)
        return (yy0 * (np.float32(s1) - dd * yy0)).astype(np.float32)

    mish_d = _register_op("MISH_D_ANT", Spec(body=body_d, reference=ref_d))

    body_o = (Src1 + C0) * (One - Src0 * C1)

    def ref_o(in0, in1, s0, s1, imm2):
        y = in0.astype(np.float32)
        z = (in1.astype(np.float32) + np.float32(s0)).astype(np.float32)
        return (z * (np.float32(1.0) - y * np.float32(s1))).astype(np.float32)

    mish_o = _register_op("MISH_OUT_ANT", Spec(body=body_o, reference=ref_o))

    # half-scale variant for the residual path: out = (z+bias)*(0.5-y)
    body_h = (Src1 + C0) * (C1 - Src0)

    def ref_h(in0, in1, s0, s1, imm2):
        y = in0.astype(np.float32)
        z = (in1.astype(np.float32) + np.float32(s0)).astype(np.float32)
        return (z * (np.float32(s1) - y)).astype(np.float32)

    mish_h = _register_op("MISH_OUTH_ANT", Spec(body=body_h, reference=ref_h))
    _MISH_OPS = (mish_d, mish_o, mish_h)
    return _MISH_OPS


def _fold_params(w1, b1, g1, be1, m1, v1, w2, b2, g2, be2, m2, v2,
                 wh, bh, gh, beh, mh, vh):
    eps = 1e-5
    f64 = np.float64
    # conv1: x is pre-scaled by 0.5 -> weights x2. BN scale folded in.
    s1 = (g1.astype(f64) / np.sqrt(v1.astype(f64) + eps))
    bv1 = ((b1.astype(f64) - m1) * s1 + be1)
    w1f = 2.0 * w1.astype(f64) * s1[:, None, None, None]  # [128, 64, 3, 3]
    # paired matmuls (dy=0, dy=1): lhsT [128, 128]: k<64 -> dy=0, k>=64 -> dy=1
    import concourse.mybir as _mybir
    bf16 = _mybir.dt.np(_mybir.dt.bfloat16)
    w1a = np.zeros((3, 128, 128), dtype=np.float32)
    for dx in range(3):
        w1a[dx, :64, :] = w1f[:, :, 1, dx].T
        w1a[dx, 64:, :] = w1f[:, :, 2, dx].T
    w1a = w1a.astype(bf16)
    # AA'-pair (dy=-1, dx=-1|0): lhsT [128, 128]
    w1c = np.zeros((1, 128, 128), dtype=np.float32)
    w1c[0, :64, :] = w1f[:, :, 0, 0].T
    w1c[0, 64:, :] = w1f[:, :, 0, 1].T
    w1c = w1c.astype(bf16)
    # single (dy=-1, dx=+1): lhsT [64, 128]
    w1b = np.zeros((1, 64, 128), dtype=np.float32)
    w1b[0, :, :] = w1f[:, :, 0, 2].T
    w1b = w1b.astype(bf16)

    s2 = (g2.astype(f64) / np.sqrt(v2.astype(f64) + eps))
    bv2 = ((b2.astype(f64) - m2) * s2 + be2)
    w2f = w2.astype(f64) * s2[:, None, None, None]  # [64, 128, 3, 3]
    # 12 matmuls indexed (s in -1..2, dx in 0..2); lhsT [128, 128]:
    #   M cols 0-63: out row r, tap dy=s; cols 64-127: out row r+1, dy=s-1
    w2t = np.zeros((12, 128, 128), dtype=np.float32)
    for si, s in enumerate((-1, 0, 1, 2)):
        for dx in range(3):
            t = si * 3 + dx
            if -1 <= s <= 1:
                w2t[t, :, 0:64] = w2f[:, :, s + 1, dx].T
            if -1 <= s - 1 <= 1:
                w2t[t, :, 64:128] = w2f[:, :, s, dx].T

    sh = (gh.astype(f64) / np.sqrt(vh.astype(f64) + eps))
    bvh = ((bh.astype(f64) - mh) * sh + beh)
    whm = wh[:, :, 0, 0].astype(f64)  # [64, 192]
    wH, wV, wD = whm[:, :64], whm[:, 64:128], whm[:, 128:]
    # Combined DWT lhsT per col-parity b: M cols 0-63 = cA (identity sum of
    # the quadrants of h2'; the folded 0.5 makes it exact), cols 64-127 =
    # x_high taps W'[o,c,a,b] = (wH*sH + wV*sV + wD*sD)*sh (x2 from h2'
    # being 0.5*h2 cancels the Haar 0.5).  lhsT[k,o]: k<64 -> a=0 (even
    # rows), k>=64 -> a=1 (odd rows).
    eye = np.eye(64, dtype=f64)
    whca = np.zeros((2, 128, 128), dtype=np.float32)
    for b in range(2):
        sV = 1.0 if b == 0 else -1.0
        wa0 = (wH * 1.0 + wV * sV + wD * sV) * sh[:, None]   # a=0: sH=+1, sD=sV
        wa1 = (wH * -1.0 + wV * sV + wD * (-sV)) * sh[:, None]  # a=1: sH=-1
        whca[b, :64, 0:64] = wa0.T
        whca[b, 64:, 0:64] = wa1.T
        whca[b, :64, 64:128] = eye
        whca[b, 64:, 64:128] = eye

    bv1 = bv1.astype(np.float32).reshape(128, 1)
    bv2d = np.tile(bv2.astype(np.float32), 2).reshape(128, 1)
    bvhd = np.tile(bvh.astype(np.float32), 2).reshape(128, 1)
    return w1a, w1b, w1c, w2t, whca, bv1, bv2d, bvhd


class _Builder:
    def __init__(self, H, W, finalize=True, reps=1, dbg=None):
        self.finalize = finalize
        self.reps = reps
        self.dbg = dbg
        import concourse.bass as bass
        import concourse.bacc as bacc
        import concourse.mybir as mybir
        from concourse.dt import dt
        from concourse.tile import TileContext
        from concourse.alu_op_type import AluOpType

        self.bass = bass
        self.bacc = bacc
        self.mybir = mybir
        self.F32, self.F32R = dt.float32, dt.float32r
        self.BF16 = dt.bfloat16
        self.Act = mybir.ActivationFunctionType
        self.Alu = AluOpType
        self.H, self.W = H, W
        self.BLOCK = 16
        self.NB = H // self.BLOCK
        self.TileContext = TileContext
        self.mq_d, self.mq_o, self.mq_h = _register_mish_ops()

    def build(self):
        H, W = self.H, self.W
        F32, F32R = self.F32, self.F32R
        Wp = W + 2
        HW2 = (H // 2) * (W // 2)
        nc = self.bacc.Bacc(None, target_bir_lowering=False)
        self.nc = nc

        BF16 = self.BF16
        self.params = {}
        for nm, shp, dtp in (
            ("w1a", [3, 128, 128], BF16), ("w1b", [1, 64, 128], BF16),
            ("w1c", [1, 128, 128], BF16),
            ("w2t", [12, 128, 128], F32R), ("whca", [2, 128, 128], F32R),
        ):
            self.params[nm] = nc.declare_dram_parameter(nm, shp, dtp,
                                                        isOutput=False)
        for nm in ("bv1", "bv2", "bvh"):
            self.params[nm] = nc.declare_dram_parameter(nm, [128, 1], F32,
                                                        isOutput=False)
        # x: host-padded+0.5-scaled: slots 0,1 zero, slot r+2 = row r,
        # slots 258..260 zero -> [64, 261*(W+2)]
        self.x = nc.declare_dram_parameter("x", [64, (H + 5) * Wp],
                                           BF16, isOutput=False)
        xlo = nc.declare_dram_parameter("x_low", [64, HW2], F32, isOutput=True)
        xhi = nc.declare_dram_parameter("x_high", [64, HW2], F32,
                                        isOutput=True)
        # per 2-supergroup unit u: 4 consecutive output rows = 512 cols
        self.xloV = xlo.rearrange("c (u z) -> c u z", z=2 * W)
        self.xhiV = xhi.rearrange("c (u z) -> c u z", z=2 * W)
        if self.dbg == "h1":
            self.dbgp = nc.declare_dram_parameter(
                "dbg", [128, self.NB * 18 * Wp], F32, isOutput=True)
        elif self.dbg == "h2":
            self.dbgp = nc.declare_dram_parameter(
                "dbg", [128, self.NB * 8 * W], F32, isOutput=True)

        with self.TileContext(nc) as tc:
            with ExitStack() as st:
                p = {}
                for name, bufs, space in (
                    ("const", 1, "SBUF"), ("ab", 2, "SBUF"),
                    ("aa", 2, "SBUF"),
                    ("h1", 2, "SBUF"), ("u", 2, "SBUF"),
                    ("y", 2, "SBUF"),
                    ("u2", 2, "SBUF"), ("y2", 2, "SBUF"),
                    ("m", 2, "SBUF"), ("h2", 2, "SBUF"),
                    ("uh", 2, "SBUF"), ("yh", 2, "SBUF"),
                    ("cat", 2, "SBUF"), ("xht", 2, "SBUF"),
                    ("ps1", 3, "PSUM"), ("ps2", 2, "PSUM"),
                    ("psh", 1, "PSUM"),
                ):
                    p[name] = st.enter_context(
                        tc.tile_pool(name=name, bufs=bufs, space=space))
                self.p = p
                self._emit_constants()
                if self.reps == 1:
                    self._emit_all()
                else:
                    with tc.For_i(0, self.reps, 1):
                        self._emit_all()
        if self.finalize:
            nc.finalize()
        return nc

    def _emit_all(self):
        for b in range(self.NB):
            self._emit_block(b)

    def _emit_constants(self):
        nc, p = self.nc, self.p
        F32, F32R = self.F32, self.F32R
        self.w1as = p["const"].tile([128, 3 * 128], self.BF16, tag="w1as")
        nc.sync.dma_start(
            out=self.w1as.rearrange("k (t m) -> k t m", m=128),
            in_=self.params["w1a"].rearrange("t k m -> k t m"))
        self.w1bs = p["const"].tile([64, 128], self.BF16, tag="w1bs")
        nc.sync.dma_start(
            out=self.w1bs.rearrange("k (t m) -> k t m", m=128),
            in_=self.params["w1b"].rearrange("t k m -> k t m"))
        self.w1cs = p["const"].tile([128, 128], self.BF16, tag="w1cs")
        nc.sync.dma_start(
            out=self.w1cs.rearrange("k (t m) -> k t m", m=128),
            in_=self.params["w1c"].rearrange("t k m -> k t m"))
        self.w2s = p["const"].tile([128, 12 * 128], F32R, tag="w2s")
        nc.sync.dma_start(
            out=self.w2s.rearrange("k (t m) -> k t m", m=128),
            in_=self.params["w2t"].rearrange("t k m -> k t m"))
        self.whs = p["const"].tile([128, 2 * 128], F32R, tag="whs")
        nc.sync.dma_start(
            out=self.whs.rearrange("k (t m) -> k t m", m=128),
            in_=self.params["whca"].rearrange("t k m -> k t m"))
        self.bv1s = p["const"].tile([128, 1], F32, tag="bv1s")
        nc.sync.dma_start(out=self.bv1s[:], in_=self.params["bv1"][:])
        self.bv2s = p["const"].tile([128, 1], F32, tag="bv2s")
        nc.sync.dma_start(out=self.bv2s[:], in_=self.params["bv2"][:])
        self.bvhs = p["const"].tile([128, 1], F32, tag="bvhs")
        nc.sync.dma_start(out=self.bvhs[:], in_=self.params["bvh"][:])

    def _emit_block(self, b):
        nc, p = self.nc, self.p
        H, W, BLOCK = self.H, self.W, self.BLOCK
        F32, F32R, Act, Alu = self.F32, self.F32R, self.Act, self.Alu
        Wp, Wh = W + 2, W // 2
        r0 = b * BLOCK

        # ---- conv1 rows: block 0 owns [-1, 16]; others [r0+1, r0+16]
        # (boundary rows r0-1, r0 come from the previous block's h1 tile) ----
        first_row = r0 - 1 if b == 0 else r0 + 1
        nrows = r0 + 17 - first_row  # 18 or 16
        # x load: A rows = x rows [first_row-1, r0+17], B rows = +1
        base = first_row + 1  # DRAM slot of x row first_row-1
        nld = nrows + 2
        ab = p["ab"].tile([128, nld * Wp], self.BF16, tag="ab")
        abv = ab.rearrange("p (r c) -> p r c", c=Wp)
        nc.sync.dma_start(out=ab[0:64, :],
                          in_=self.x[:, base * Wp:(base + nld) * Wp])
        nc.sync.dma_start(out=ab[64:128, :],
                          in_=self.x[:, (base + 1) * Wp:(base + nld + 1) * Wp])
        aa = p["aa"].tile([128, nld * Wp], self.BF16, tag="aa")
        aav = aa.rearrange("p (r c) -> p r c", c=Wp)
        nc.sync.dma_start(out=aa[0:64, :],
                          in_=self.x[:, base * Wp:(base + nld) * Wp])
        nc.sync.dma_start(out=aa[64:128, :],
                          in_=self.x[:, base * Wp + 1:(base + nld) * Wp + 1])

        h1 = p["h1"].tile([128, nrows * Wp + 4], F32R, tag="h1")
        h1v = h1[:, 0:nrows * Wp].rearrange("p (r c) -> p r c", c=Wp)
        h1f = h1.bitcast(F32)
        h1fv = h1f[:, 0:nrows * Wp].rearrange("p (r c) -> p r c", c=Wp)
        nc.vector.memset(h1fv[:, :, 0:1], 0.0)
        nc.vector.memset(h1fv[:, :, W + 1:W + 2], 0.0)
        for gi in range(nrows // 2):
            a = first_row + 2 * gi  # image rows (a, a+1)
            ti = a - first_row + 1  # A tile row of x row a
            psum = p["ps1"].tile([128, 2 * W], F32, tag="ps1")
            pv = psum.rearrange("p (r c) -> p r c", c=W)
            mms = []
            for dx in range(3):
                # pair (dy=0, dy=1): A rows (a, a+1), B rows (a+1, a+2)
                mms.append((pv[:, :, :], self.w1as[:, dx * 128:(dx + 1) * 128],
                            abv[:, ti:ti + 2, dx:dx + W]))
            # pair (dy=-1, dx=-1|0) on the [x ; x+1col] tile
            mms.append((pv[:, :, :], self.w1cs[:, 0:128],
                        aav[:, ti - 1:ti + 1, 0:W]))
            # single (dy=-1, dx=+1)
            mms.append((pv[:, :, :], self.w1bs[:, 0:128],
                        abv[0:64, ti - 1:ti + 1, 2:2 + W]))
            for i, (o, l, rr) in enumerate(mms):
                nc.tensor.matmul(o, l, rr, start=(i == 0), stop=(i == 4))
            u = p["u"].tile([128, 2 * W], F32, tag="u")
            nc.scalar.activation(u[:], psum[:], Act.Exp, bias=self.bv1s[:])
            y = p["y"].tile([128, 2 * W], F32, tag="y")
            nc.vector._custom_dve(self.mq_d, out=y[:], in0=u[:],
                                  s0=MQ_C0, s1=MQ_C1, imm2=2.0)
            lr = a - first_row  # h1 tile row
            yv = y.rearrange("p (r c) -> p r c", c=W)
            nc.vector._custom_dve(self.mq_o, out=h1v[:, lr:lr + 2, 1:W + 1],
                                  in0=yv, in1=pv, s0=self.bv1s[:], s1=2.0)
        if b == 0:
            nc.vector.memset(h1fv[:, 0:1, :], 0.0)   # h1 row -1 := 0
        if b == self.NB - 1:
            nc.vector.memset(h1fv[:, nrows - 1:nrows, :], 0.0)  # row H := 0

        # ---- conv2 + mish + residual -> h2' per supergroup (4 rows) ----
        h2 = p["h2"].tile([128, (BLOCK // 2) * W], F32R, tag="h2")
        h2f = h2.bitcast(F32)
        # deinterleaved write view: (pair, j, parity)
        h2w = h2f.rearrange("p (sg pr pp j) -> p sg pr j pp",
                            sg=4, pr=2, pp=2, j=Wh)
        # matmul read view: (sg, pair, parity, j)
        h2r = h2.rearrange("p (sg pr pp j) -> p sg pr pp j",
                           sg=4, pr=2, pp=2, j=Wh)
        # even-row-pair view of x for the residual: row = 2*hh + two
        abp = ab.rearrange("p (hh two c) -> p hh two c", two=2, c=Wp)
        for sg in range(4):
            r = r0 + 4 * sg
            psum = p["ps2"].tile([128, 1024], F32, tag="ps2")
            for pr in range(2):
                rr = r + 2 * pr
                pbase = 512 * pr
                for si, s in enumerate((0, -1, 1, 2)):
                    row = rr + s
                    if row >= first_row:
                        srct, hrow = h1, row - first_row
                    else:
                        srct, hrow = self.h1_prev, row - self.h1_prev_first
                    for dxi, dx in enumerate((1, 0, -1) if si == 0
                                             else (0, 1, -1)):
                        first = (si == 0 and dxi == 0)
                        # fp32r psum dst must be 2-element aligned: dx=0
                        # reads the row from col 1 so its dst lands at 2.
                        rs = 1 if dx == 0 else 0
                        off = 1 - dx + rs
                        wid = 260 if first else 258
                        t = (1 + s) * 3 + (dx + 1)
                        nc.tensor.matmul(
                            psum[:, pbase + off:pbase + off + wid],
                            self.w2s[:, t * 128:(t + 1) * 128],
                            srct[:, hrow * Wp + rs:hrow * Wp + rs + wid],
                            start=first, stop=(si == 3 and dxi == 2),
                            skip_group_check=True)
            pval = psum.rearrange("p (pr c) -> p pr c", c=512)[:, :, 2:2 + W]
            u2 = p["u2"].tile([128, 2 * W], F32, tag="u2")
            u2v = u2.rearrange("p (pr c) -> p pr c", c=W)
            nc.scalar.activation(u2v, pval, Act.Exp, bias=self.bv2s[:])
            y2 = p["y2"].tile([128, 2 * W], F32, tag="y2")
            nc.vector._custom_dve(self.mq_d, out=y2[:], in0=u2[:],
                                  s0=MQ_C0, s1=MQ_C1, imm2=2.0)
            m = p["m"].tile([128, 2 * W], F32, tag="m")
            mv = m.rearrange("p (pr c) -> p pr c", c=W)
            y2v = y2.rearrange("p (pr c) -> p pr c", c=W)
            nc.vector._custom_dve(self.mq_h, out=mv, in0=y2v, in1=pval,
                                  s0=self.bv2s[:], s1=0.5)
            # residual: h2' = 0.5*m + x_sbuf; x rows (r, r+2 | r+1, r+3)
            # = AB tile rows (i, i+2), i = r+2-r0 (always even).  stt out
            # APs allow <=2 free dims, so one op per pair, deinterleaving
            # [j, parity] on the write.
            i2 = (r + 2 - first_row) // 2  # x row r+2pr -> abp[hh]
            for pr in range(2):
                xres = abp[:, i2 + pr, 0, 1:W + 1].rearrange(
                    "p (j pp) -> p j pp", pp=2)
                mpr = m[:, pr * W:(pr + 1) * W].rearrange(
                    "p (j pp) -> p j pp", pp=2)
                hout = h2[:, sg * 2 * W + pr * W:
                          sg * 2 * W + (pr + 1) * W].rearrange(
                    "p (pp j) -> p j pp", pp=2, j=Wh)
                nc.gpsimd.tensor_add(out=hout, in0=mpr, in1=xres)

        # ---- DWT + convh: one M-packed matmul pair per supergroup
        # (out parts 0-63 = cA, 64-127 = x_high); evac per 2 supergroups ----
        for su in range(2):
            psd = p["psh"].tile([128, 4 * Wh], F32, tag="psh")
            for k in range(2):
                sg = 2 * su + k
                pdv = psd[:, k * 2 * Wh:(k + 1) * 2 * Wh].rearrange(
                    "p (pr j) -> p pr j", j=Wh)
                for bb in range(2):
                    nc.tensor.matmul(pdv, self.whs[:, bb * 128:(bb + 1) * 128],
                                     h2r[:, sg, :, bb, :],
                                     start=(bb == 0), stop=(bb == 1),
                                     skip_group_check=True)
            uh = p["uh"].tile([64, 4 * Wh], F32, tag="uh")
            nc.scalar.activation(uh[:], psd[0:64, :], Act.Exp,
                                 bias=self.bvhs[0:64])
            yh = p["yh"].tile([64, 4 * Wh], F32, tag="yh")
            nc.vector._custom_dve(self.mq_d, out=yh[:], in0=uh[:],
                                  s0=MQ_C0, s1=MQ_C1, imm2=2.0)
            xht = p["xht"].tile([64, 4 * Wh], F32, tag="xht")
            nc.vector._custom_dve(self.mq_o, out=xht[:], in0=yh[:],
                                  in1=psd[0:64, :], s0=self.bvhs[0:64],
                                  s1=2.0)
            cat = p["cat"].tile([128, 4 * Wh], F32, tag="cat")
            nc.scalar.activation(cat[64:128, :], psd[64:128, :], Act.Identity)
            uu = 2 * b + su
            nc.sync.dma_start(out=self.xloV[:, uu, :], in_=cat[64:128, :])
            nc.sync.dma_start(out=self.xhiV[:, uu, :], in_=xht[:])
        self.h1_prev, self.h1_prev_first = h1, first_row
        if self.dbg == "h1":
            nc.sync.dma_start(
                out=self.dbgp[:, b * 18 * Wp:(b + 1) * 18 * Wp],
                in_=h1f[:, 0:18 * Wp])
        elif self.dbg == "h2":
            nc.sync.dma_start(
                out=self.dbgp[:, b * 8 * W:(b + 1) * 8 * W], in_=h2f[:])


def _build(H, W, finalize=True, reps=1, dbg=None):
    return _Builder(H, W, finalize=finalize, reps=reps, dbg=dbg).build()


def _get_program(H, W, reps=1):
    key = (H, W, reps)
    if key not in _CACHE:
        _CACHE[key] = _build(H, W, reps=reps)
    return _CACHE[key]


def _prep_inputs(x, w1, b1, g1, be1, m1, v1, w2, b2, g2, be2, m2, v2,
                 wh, bh, gh, beh, mh, vh):
    x = np.asarray(x, dtype=np.float32)
    B, C, H, W = x.shape
    w1a, w1b, w1c, w2t, whca, bv1, bv2d, bvhd = _fold_params(
        np.asarray(w1, np.float32), np.asarray(b1, np.float32),
        np.asarray(g1, np.float32), np.asarray(be1, np.float32),
        np.asarray(m1, np.float32), np.asarray(v1, np.float32),
        np.asarray(w2, np.float32), np.asarray(b2, np.float32),
        np.asarray(g2, np.float32), np.asarray(be2, np.float32),
        np.asarray(m2, np.float32), np.asarray(v2, np.float32),
        np.asarray(wh, np.float32), np.asarray(bh, np.float32),
        np.asarray(gh, np.float32), np.asarray(beh, np.float32),
        np.asarray(mh, np.float32), np.asarray(vh, np.float32))
    import concourse.mybir as _mybir
    bf16 = _mybir.dt.np(_mybir.dt.bfloat16)
    xp = np.zeros((B, C, H + 5, W + 2), dtype=bf16)
    xp[:, :, 2:H + 2, 1:W + 1] = (0.5 * x).astype(bf16)
    in_maps = []
    for i in range(B):
        in_maps.append({
            "x": np.ascontiguousarray(xp[i].reshape(C, (H + 5) * (W + 2))),
            "w1a": w1a, "w1b": w1b, "w1c": w1c, "w2t": w2t, "whca": whca,
            "bv1": bv1, "bv2": bv2d, "bvh": bvhd,
        })
    return in_maps, B, C, H, W


def kernel(x, w1, b1, g1, be1, m1, v1, w2, b2, g2, be2, m2, v2,
           wh, bh, gh, beh, mh, vh):
    from concourse.bass_utils import run_bass_kernel_spmd

    in_maps, B, C, H, W = _prep_inputs(
        x, w1, b1, g1, be1, m1, v1, w2, b2, g2, be2, m2, v2,
        wh, bh, gh, beh, mh, vh)
    nc = _get_program(H, W)
    res = run_bass_kernel_spmd(nc, in_maps, list(range(B)), trace=False)
    if res.exec_time_ns is not None:
        print(f"HW exec time: {res.exec_time_ns} ns")
    H2, W2 = H // 2, W // 2
    x_low = np.stack([res.results[i]["x_low"].astype(np.float32)
                      .reshape(C, H2, W2) for i in range(B)])
    x_high = np.stack([res.results[i]["x_high"].astype(np.float32)
                       .reshape(C, H2, W2) for i in range(B)])
    return (x_low, x_high)


# revision 44
# speedup vs baseline: 2.2630x; 1.0745x over previous
"""Trainium2 Bass kernel for nn_FSE_Module_79147657331158.

Pipeline (one image per NeuronCore, 8-way batch data parallel):
  h1 = mish(BN1(conv3x3(x, w1)))          64 -> 128 ch
  h2 = mish(BN2(conv3x3(h1, w2))) + x     128 -> 64 ch
  x_low  = cA(haar_dwt2(h2))
  x_high = mish(BNh(conv1x1(concat(cH,cV,cD), wh)))

~394 us / batch on 8 cores (baseline 891 us; 2.26x).  Key ideas:
  - mish in 1 ACT + 2 custom-DVE ops per element: u = exp(z+b) [ACT],
    MISH_D computes y ~= 1/(u(u+2)+2) in one DVE instruction (bitwise-NOT
    reciprocal seed + 1 Newton step, 8 ALU stages), MISH_OUT computes
    (z+b)*(1-2y) streaming z straight from PSUM with the per-partition
    bias in a scalar slot (mish(z) = z*(g-1)/(g+1), g = (1+e^z)^2, and
    (g-1)/(g+1) = 1-2y).  vs 3 ACT + 3 DVE ops for the unfused chain.
  - conv1 in 5 matmuls per 2-row group (vs 9): dy-taps K=128-packed on
    [x ; x+1row] and [x ; x+1col] dual tiles (x, w1 in bf16).
  - conv2: two output rows packed in the PE's M dim (out chans 64).
    All taps stream FULL h1 rows (F=258) and land at per-dx column
    offsets in a [128, 260] psum window; psum itself does the dx
    re-alignment (fp32r psum dst must be 2-elem aligned, so the dx=0
    tap reads rows from col 1).  12 matmuls x F=258 per row pair =
    1.49x fewer PE cycles than 9 x F=512, and the mish runs on 128
    partitions.
  - residual on the otherwise-idle GPSIMD engine as a plain tensor_add
    (h2' = 0.5*mish2 + 0.5*x; x pre-scaled 0.5 on host, w1 x2, conv2
    out op emits 0.5*mish via a (z+b)*(0.5-y) variant).
  - DWT: h2' holds even rows on partitions 0-63, odd on 64-127, so the
    row-parity taps K-pack to K=128, and cA + x_high share each matmul's
    M halves: 2 matmuls of F=256 per supergroup total.
  - h1 is a sliding window: blocks read boundary rows from the previous
    block's tile (pool bufs=2), no halo recompute.
  - custom DVE ops NEVER run at partition offset 64 (silently broken
    there); ACT/stt are fine at offset 64.
"""
import os
import sys
from contextlib import ExitStack

sys.path.insert(0, "/opt/trn_rl_repo")

import numpy as np

_CACHE = {}

MQ_C0 = -0.23549792
MQ_C1 = 2.0017324
_MISH_OPS = None


def _register_op(name, spec):
    import concourse.dve_ops as dve_ops
    from concourse.dve_spec import lower, _has_src1
    from concourse.dve_uop import DveOpSpec

    for op in dve_ops.OPS:
        if op.name == name:
            return op
    row = max(dve_ops._SUB_OPCODE_FOR_NAME.values()) + 1
    assert row < 0x20
    dve_ops._SUB_OPCODE_FOR_NAME[name] = row
    shas = {}
    for ver in ("v3", "v4"):
        s = DveOpSpec(name=name, opcode=row, uops=lower(spec, ver=ver),
                      rd1_en=_has_src1(spec))
        shas[ver] = s.sha(ver)
    op = dve_ops.DveOp(name, spec, subdim=False, uops_sha=shas)
    dve_ops.OPS.append(op)
    dve_ops.CUSTOM_DVE_SPECS[name] = spec
    return op


def _register_mish_ops():
    """MISH_D: y ~= 1/(u(u+2)+2) (NOT-seed + 1 Newton step, 8 stages).
    MISH_OUT: out = (z + bias)*(1 - 2y) with z streamed from PSUM and the
    per-partition bias riding the s0 scalar slot (4 stages).
    mish(z) = z*(g-1)/(g+1), g=(1+e^z)^2 = u(u+2)+1, so (g-1)/(g+1)=1-2y.
    """
    global _MISH_OPS
    if _MISH_OPS is not None:
        return _MISH_OPS
    from concourse.dve_spec import Spec, Src0, Src1, C0, C1, C2, One, AluOp, Bin

    n = (Src0 + C2) * Src0
    d = n + C2
    nd = Bin(AluOp.BITWISE_NOT, d, d)
    y0 = nd * C0
    body_d = y0 * (C1 - d * y0)

    def ref_d(in0, in1, s0, s1, imm2):
        u = np.ascontiguousarray(in0.astype(np.float32))
        nn = ((u + np.float32(imm2)) * u).astype(np.float32)
        dd = (nn + np.float32(imm2)).astype(np.float32)
        ndv = (~dd.view(np.int32)).view(np.float32)
        yy0 = (ndv * np.float32(s0)).astype(np.float32)
        return (yy0 * (np.float32(s1) - dd * yy0)).astype(np.float32)

    mish_d = _register_op("MISH_D_ANT", Spec(body=body_d, reference=ref_d))

    body_o = (Src1 + C0) * (One - Src0 * C1)

    def ref_o(in0, in1, s0, s1, imm2):
        y = in0.astype(np.float32)
        z = (in1.astype(np.float32) + np.float32(s0)).astype(np.float32)
        return (z * (np.float32(1.0) - y * np.float32(s1))).astype(np.float32)

    mish_o = _register_op("MISH_OUT_ANT", Spec(body=body_o, reference=ref_o))

    # half-scale variant for the residual path: out = (z+bias)*(0.5-y)
    body_h = (Src1 + C0) * (C1 - Src0)

    def ref_h(in0, in1, s0, s1, imm2):
        y = in0.astype(np.float32)
        z = (in1.astype(np.float32) + np.float32(s0)).astype(np.float32)
        return (z * (np.float32(s1) - y)).astype(np.float32)

    mish_h = _register_op("MISH_OUTH_ANT", Spec(body=body_h, reference=ref_h))
    _MISH_OPS = (mish_d, mish_o, mish_h)
    return _MISH_OPS


def _fold_params(w1, b1, g1, be1, m1, v1, w2, b2, g2, be2, m2, v2,
                 wh, bh, gh, beh, mh, vh):
    eps = 1e-5
    f64 = np.float64
    # conv1: x is pre-scaled by 0.5 -> weights x2. BN scale folded in.
    s1 = (g1.astype(f64) / np.sqrt(v1.astype(f64) + eps))
    bv1 = ((b1.astype(f64) - m1) * s1 + be1)
    w1f = 2.0 * w1.astype(f64) * s1[:, None, None, None]  # [128, 64, 3, 3]
    # paired matmuls (dy=0, dy=1): lhsT [128, 128]: k<64 -> dy=0, k>=64 -> dy=1
    import concourse.mybir as _mybir
    bf16 = _mybir.dt.np(_mybir.dt.bfloat16)
    w1a = np.zeros((3, 128, 128), dtype=np.float32)
    for dx in range(3):
        w1a[dx, :64, :] = w1f[:, :, 1, dx].T
        w1a[dx, 64:, :] = w1f[:, :, 2, dx].T
    w1a = w1a.astype(bf16)
    # AA'-pair (dy=-1, dx=-1|0): lhsT [128, 128]
    w1c = np.zeros((1, 128, 128), dtype=np.float32)
    w1c[0, :64, :] = w1f[:, :, 0, 0].T
    w1c[0, 64:, :] = w1f[:, :, 0, 1].T
    w1c = w1c.astype(bf16)
    # single (dy=-1, dx=+1): lhsT [64, 128]
    w1b = np.zeros((1, 64, 128), dtype=np.float32)
    w1b[0, :, :] = w1f[:, :, 0, 2].T
    w1b = w1b.astype(bf16)

    s2 = (g2.astype(f64) / np.sqrt(v2.astype(f64) + eps))
    bv2 = ((b2.astype(f64) - m2) * s2 + be2)
    w2f = w2.astype(f64) * s2[:, None, None, None]  # [64, 128, 3, 3]
    # 12 matmuls indexed (s in -1..2, dx in 0..2); lhsT [128, 128]:
    #   M cols 0-63: out row r, tap dy=s; cols 64-127: out row r+1, dy=s-1
    w2t = np.zeros((12, 128, 128), dtype=np.float32)
    for si, s in enumerate((-1, 0, 1, 2)):
        for dx in range(3):
            t = si * 3 + dx
            if -1 <= s <= 1:
                w2t[t, :, 0:64] = w2f[:, :, s + 1, dx].T
            if -1 <= s - 1 <= 1:
                w2t[t, :, 64:128] = w2f[:, :, s, dx].T

    sh = (gh.astype(f64) / np.sqrt(vh.astype(f64) + eps))
    bvh = ((bh.astype(f64) - mh) * sh + beh)
    whm = wh[:, :, 0, 0].astype(f64)  # [64, 192]
    wH, wV, wD = whm[:, :64], whm[:, 64:128], whm[:, 128:]
    # Combined DWT lhsT per col-parity b: M cols 0-63 = cA (identity sum of
    # the quadrants of h2'; the folded 0.5 makes it exact), cols 64-127 =
    # x_high taps W'[o,c,a,b] = (wH*sH + wV*sV + wD*sD)*sh (x2 from h2'
    # being 0.5*h2 cancels the Haar 0.5).  lhsT[k,o]: k<64 -> a=0 (even
    # rows), k>=64 -> a=1 (odd rows).
    eye = np.eye(64, dtype=f64)
    whca = np.zeros((2, 128, 128), dtype=np.float32)
    for b in range(2):
        sV = 1.0 if b == 0 else -1.0
        wa0 = (wH * 1.0 + wV * sV + wD * sV) * sh[:, None]   # a=0: sH=+1, sD=sV
        wa1 = (wH * -1.0 + wV * sV + wD * (-sV)) * sh[:, None]  # a=1: sH=-1
        whca[b, :64, 0:64] = wa0.T
        whca[b, 64:, 0:64] = wa1.T
        whca[b, :64, 64:128] = eye
        whca[b, 64:, 64:128] = eye

    bv1 = bv1.astype(np.float32).reshape(128, 1)
    bv2d = np.tile(bv2.astype(np.float32), 2).reshape(128, 1)
    bvhd = np.tile(bvh.astype(np.float32), 2).reshape(128, 1)
    return w1a, w1b, w1c, w2t, whca, bv1, bv2d, bvhd


class _Builder:
    def __init__(self, H, W, finalize=True, reps=1, dbg=None):
        self.finalize = finalize
        self.reps = reps
        self.dbg = dbg
        import concourse.bass as bass
        import concourse.bacc as bacc
        import concourse.mybir as mybir
        from concourse.dt import dt
        from concourse.tile import TileContext
        from concourse.alu_op_type import AluOpType

        self.bass = bass
        self.bacc = bacc
        self.mybir = mybir
        self.F32, self.F32R = dt.float32, dt.float32r
        self.BF16 = dt.bfloat16
        self.Act = mybir.ActivationFunctionType
        self.Alu = AluOpType
        self.H, self.W = H, W
        self.BLOCK = 16
        self.NB = H // self.BLOCK
        self.TileContext = TileContext
        self.mq_d, self.mq_o, self.mq_h = _register_mish_ops()

    def build(self):
        H, W = self.H, self.W
        F32, F32R = self.F32, self.F32R
        Wp = W + 2
        HW2 = (H // 2) * (W // 2)
        nc = self.bacc.Bacc(None, target_bir_lowering=False)
        self.nc = nc

        BF16 = self.BF16
        self.params = {}
        for nm, shp, dtp in (
            ("w1a", [3, 128, 128], BF16), ("w1b", [1, 64, 128], BF16),
            ("w1c", [1, 128, 128], BF16),
            ("w2t", [12, 128, 128], F32R), ("whca", [2, 128, 128], F32R),
        ):
            self.params[nm] = nc.declare_dram_parameter(nm, shp, dtp,
                                                        isOutput=False)
        for nm in ("bv1", "bv2", "bvh"):
            self.params[nm] = nc.declare_dram_parameter(nm, [128, 1], F32,
                                                        isOutput=False)
        # x: host-padded+0.5-scaled: slots 0,1 zero, slot r+2 = row r,
        # slots 258..260 zero -> [64, 261*(W+2)]
        self.x = nc.declare_dram_parameter("x", [64, (H + 5) * Wp],
                                           BF16, isOutput=False)
        xlo = nc.declare_dram_parameter("x_low", [64, HW2], F32, isOutput=True)
        xhi = nc.declare_dram_parameter("x_high", [64, HW2], F32,
                                        isOutput=True)
        # per 2-supergroup unit u: 4 consecutive output rows = 512 cols
        self.xloV = xlo.rearrange("c (u z) -> c u z", z=2 * W)
        self.xhiV = xhi.rearrange("c (u z) -> c u z", z=2 * W)
        if self.dbg == "h1":
            self.dbgp = nc.declare_dram_parameter(
                "dbg", [128, self.NB * 18 * Wp], F32, isOutput=True)
        elif self.dbg == "h2":
            self.dbgp = nc.declare_dram_parameter(
                "dbg", [128, self.NB * 8 * W], F32, isOutput=True)

        with self.TileContext(nc) as tc:
            with ExitStack() as st:
                p = {}
                for name, bufs, space in (
                    ("const", 1, "SBUF"), ("ab", 2, "SBUF"),
                    ("aa", 2, "SBUF"),
                    ("h1", 2, "SBUF"), ("u", 2, "SBUF"),
                    ("y", 2, "SBUF"),
                    ("u2", 2, "SBUF"), ("y2", 2, "SBUF"),
                    ("m", 2, "SBUF"), ("h2", 2, "SBUF"),
                    ("uh", 2, "SBUF"), ("yh", 2, "SBUF"),
                    ("cat", 2, "SBUF"), ("xht", 2, "SBUF"),
                    ("ps1", 3, "PSUM"), ("ps2", 2, "PSUM"),
                    ("psh", 1, "PSUM"),
                ):
                    p[name] = st.enter_context(
                        tc.tile_pool(name=name, bufs=bufs, space=space))
                self.p = p
                self._emit_constants()
                if self.reps == 1:
                    self._emit_all()
                else:
                    with tc.For_i(0, self.reps, 1):
                        self._emit_all()
        if self.finalize:
            nc.finalize()
        return nc

    def _emit_all(self):
        for b in range(self.NB):
            self._emit_block(b)

    def _emit_constants(self):
        nc, p = self.nc, self.p
        F32, F32R = self.F32, self.F32R
        self.w1as = p["const"].tile([128, 3 * 128], self.BF16, tag="w1as")
        nc.sync.dma_start(
            out=self.w1as.rearrange("k (t m) -> k t m", m=128),
            in_=self.params["w1a"].rearrange("t k m -> k t m"))
        self.w1bs = p["const"].tile([64, 128], self.BF16, tag="w1bs")
        nc.sync.dma_start(
            out=self.w1bs.rearrange("k (t m) -> k t m", m=128),
            in_=self.params["w1b"].rearrange("t k m -> k t m"))
        self.w1cs = p["const"].tile([128, 128], self.BF16, tag="w1cs")
        nc.sync.dma_start(
            out=self.w1cs.rearrange("k (t m) -> k t m", m=128),
            in_=self.params["w1c"].rearrange("t k m -> k t m"))
        self.w2s = p["const"].tile([128, 12 * 128], F32R, tag="w2s")
        nc.sync.dma_start(
            out=self.w2s.rearrange("k (t m) -> k t m", m=128),
            in_=self.params["w2t"].rearrange("t k m -> k t m"))
        self.whs = p["const"].tile([128, 2 * 128], F32R, tag="whs")
        nc.sync.dma_start(
            out=self.whs.rearrange("k (t m) -> k t m", m=128),
            in_=self.params["whca"].rearrange("t k m -> k t m"))
        self.bv1s = p["const"].tile([128, 1], F32, tag="bv1s")
        nc.sync.dma_start(out=self.bv1s[:], in_=self.params["bv1"][:])
        self.bv2s = p["const"].tile([128, 1], F32, tag="bv2s")
        nc.sync.dma_start(out=self.bv2s[:], in_=self.params["bv2"][:])
        self.bvhs = p["const"].tile([128, 1], F32, tag="bvhs")
        nc.sync.dma_start(out=self.bvhs[:], in_=self.params["bvh"][:])

    def _emit_block(self, b):
        nc, p = self.nc, self.p
        H, W, BLOCK = self.H, self.W, self.BLOCK
        F32, F32R, Act, Alu = self.F32, self.F32R, self.Act, self.Alu
        Wp, Wh = W + 2, W // 2
        r0 = b * BLOCK

        # ---- conv1 rows: block 0 owns [-1, 16]; others [r0+1, r0+16]
        # (boundary rows r0-1, r0 come from the previous block's h1 tile) ----
        first_row = r0 - 1 if b == 0 else r0 + 1
        nrows = r0 + 17 - first_row  # 18 or 16
        # x load: A rows = x rows [first_row-1, r0+17], B rows = +1
        base = first_row + 1  # DRAM slot of x row first_row-1
        nld = nrows + 2
        ab = p["ab"].tile([128, nld * Wp], self.BF16, tag="ab")
        abv = ab.rearrange("p (r c) -> p r c", c=Wp)
        nc.sync.dma_start(out=ab[0:64, :],
                          in_=self.x[:, base * Wp:(base + nld) * Wp])
        nc.scalar.dma_start(out=ab[64:128, :],
                            in_=self.x[:, (base + 1) * Wp:
                                       (base + nld + 1) * Wp])
        aa = p["aa"].tile([128, nld * Wp], self.BF16, tag="aa")
        aav = aa.rearrange("p (r c) -> p r c", c=Wp)
        nc.sync.dma_start(out=aa[0:64, :],
                          in_=self.x[:, base * Wp:(base + nld) * Wp])
        nc.scalar.dma_start(out=aa[64:128, :],
                            in_=self.x[:, base * Wp + 1:
                                       (base + nld) * Wp + 1])

        h1 = p["h1"].tile([128, nrows * Wp + 4], F32R, tag="h1")
        h1v = h1[:, 0:nrows * Wp].rearrange("p (r c) -> p r c", c=Wp)
        h1f = h1.bitcast(F32)
        h1fv = h1f[:, 0:nrows * Wp].rearrange("p (r c) -> p r c", c=Wp)
        nc.vector.memset(h1fv[:, :, 0:1], 0.0)
        nc.vector.memset(h1fv[:, :, W + 1:W + 2], 0.0)
        for gi in range(nrows // 2):
            a = first_row + 2 * gi  # image rows (a, a+1)
            ti = a - first_row + 1  # A tile row of x row a
            psum = p["ps1"].tile([128, 2 * W], F32, tag="ps1")
            pv = psum.rearrange("p (r c) -> p r c", c=W)
            mms = []
            for dx in range(3):
                # pair (dy=0, dy=1): A rows (a, a+1), B rows (a+1, a+2)
                mms.append((pv[:, :, :], self.w1as[:, dx * 128:(dx + 1) * 128],
                            abv[:, ti:ti + 2, dx:dx + W]))
            # pair (dy=-1, dx=-1|0) on the [x ; x+1col] tile
            mms.append((pv[:, :, :], self.w1cs[:, 0:128],
                        aav[:, ti - 1:ti + 1, 0:W]))
            # single (dy=-1, dx=+1)
            mms.append((pv[:, :, :], self.w1bs[:, 0:128],
                        abv[0:64, ti - 1:ti + 1, 2:2 + W]))
            for i, (o, l, rr) in enumerate(mms):
                nc.tensor.matmul(o, l, rr, start=(i == 0), stop=(i == 4))
            u = p["u"].tile([128, 2 * W], F32, tag="u")
            nc.scalar.activation(u[:], psum[:], Act.Exp, bias=self.bv1s[:])
            y = p["y"].tile([128, 2 * W], F32, tag="y")
            nc.vector._custom_dve(self.mq_d, out=y[:], in0=u[:],
                                  s0=MQ_C0, s1=MQ_C1, imm2=2.0)
            lr = a - first_row  # h1 tile row
            yv = y.rearrange("p (r c) -> p r c", c=W)
            nc.vector._custom_dve(self.mq_o, out=h1v[:, lr:lr + 2, 1:W + 1],
                                  in0=yv, in1=pv, s0=self.bv1s[:], s1=2.0)
        if b == 0:
            nc.vector.memset(h1fv[:, 0:1, :], 0.0)   # h1 row -1 := 0
        if b == self.NB - 1:
            nc.vector.memset(h1fv[:, nrows - 1:nrows, :], 0.0)  # row H := 0

        # ---- conv2 + mish + residual -> h2' per supergroup (4 rows) ----
        h2 = p["h2"].tile([128, (BLOCK // 2) * W], F32R, tag="h2")
        h2f = h2.bitcast(F32)
        # deinterleaved write view: (pair, j, parity)
        h2w = h2f.rearrange("p (sg pr pp j) -> p sg pr j pp",
                            sg=4, pr=2, pp=2, j=Wh)
        # matmul read view: (sg, pair, parity, j)
        h2r = h2.rearrange("p (sg pr pp j) -> p sg pr pp j",
                           sg=4, pr=2, pp=2, j=Wh)
        # even-row-pair view of x for the residual: row = 2*hh + two
        abp = ab.rearrange("p (hh two c) -> p hh two c", two=2, c=Wp)
        for sg in range(4):
            r = r0 + 4 * sg
            psum = p["ps2"].tile([128, 1024], F32, tag="ps2")
            for pr in range(2):
                rr = r + 2 * pr
                pbase = 512 * pr
                for si, s in enumerate((0, -1, 1, 2)):
                    row = rr + s
                    if row >= first_row:
                        srct, hrow = h1, row - first_row
                    else:
                        srct, hrow = self.h1_prev, row - self.h1_prev_first
                    for dxi, dx in enumerate((1, 0, -1) if si == 0
                                             else (0, 1, -1)):
                        first = (si == 0 and dxi == 0)
                        # fp32r psum dst must be 2-element aligned: dx=0
                        # reads the row from col 1 so its dst lands at 2.
                        rs = 1 if dx == 0 else 0
                        off = 1 - dx + rs
                        wid = 260 if first else 258
                        t = (1 + s) * 3 + (dx + 1)
                        nc.tensor.matmul(
                            psum[:, pbase + off:pbase + off + wid],
                            self.w2s[:, t * 128:(t + 1) * 128],
                            srct[:, hrow * Wp + rs:hrow * Wp + rs + wid],
                            start=first, stop=(si == 3 and dxi == 2),
                            skip_group_check=True)
            pval = psum.rearrange("p (pr c) -> p pr c", c=512)[:, :, 2:2 + W]
            u2 = p["u2"].tile([128, 2 * W], F32, tag="u2")
            u2v = u2.rearrange("p (pr c) -> p pr c", c=W)
            nc.scalar.activation(u2v, pval, Act.Exp, bias=self.bv2s[:])
            y2 = p["y2"].tile([128, 2 * W], F32, tag="y2")
            nc.vector._custom_dve(self.mq_d, out=y2[:], in0=u2[:],
                                  s0=MQ_C0, s1=MQ_C1, imm2=2.0)
            m = p["m"].tile([128, 2 * W], F32, tag="m")
            mv = m.rearrange("p (pr c) -> p pr c", c=W)
            y2v = y2.rearrange("p (pr c) -> p pr c", c=W)
            nc.vector._custom_dve(self.mq_h, out=mv, in0=y2v, in1=pval,
                                  s0=self.bv2s[:], s1=0.5)
            # residual: h2' = 0.5*m + x_sbuf; x rows (r, r+2 | r+1, r+3)
            # = AB tile rows (i, i+2), i = r+2-r0 (always even).  stt out
            # APs allow <=2 free dims, so one op per pair, deinterleaving
            # [j, parity] on the write.
            i2 = (r + 2 - first_row) // 2  # x row r+2pr -> abp[hh]
            for pr in range(2):
                xres = abp[:, i2 + pr, 0, 1:W + 1].rearrange(
                    "p (j pp) -> p j pp", pp=2)
                mpr = m[:, pr * W:(pr + 1) * W].rearrange(
                    "p (j pp) -> p j pp", pp=2)
                hout = h2[:, sg * 2 * W + pr * W:
                          sg * 2 * W + (pr + 1) * W].rearrange(
                    "p (pp j) -> p j pp", pp=2, j=Wh)
                nc.gpsimd.tensor_add(out=hout, in0=mpr, in1=xres)

        # ---- DWT + convh: one M-packed matmul pair per supergroup
        # (out parts 0-63 = cA, 64-127 = x_high); evac per 2 supergroups ----
        for su in range(2):
            psd = p["psh"].tile([128, 4 * Wh], F32, tag="psh")
            for k in range(2):
                sg = 2 * su + k
                pdv = psd[:, k * 2 * Wh:(k + 1) * 2 * Wh].rearrange(
                    "p (pr j) -> p pr j", j=Wh)
                for bb in range(2):
                    nc.tensor.matmul(pdv, self.whs[:, bb * 128:(bb + 1) * 128],
                                     h2r[:, sg, :, bb, :],
                                     start=(bb == 0), stop=(bb == 1),
                                     skip_group_check=True)
            uh = p["uh"].tile([64, 4 * Wh], F32, tag="uh")
            nc.scalar.activation(uh[:], psd[0:64, :], Act.Exp,
                                 bias=self.bvhs[0:64])
            yh = p["yh"].tile([64, 4 * Wh], F32, tag="yh")
            nc.vector._custom_dve(self.mq_d, out=yh[:], in0=uh[:],
                                  s0=MQ_C0, s1=MQ_C1, imm2=2.0)
            xht = p["xht"].tile([64, 4 * Wh], F32, tag="xht")
            nc.vector._custom_dve(self.mq_o, out=xht[:], in0=yh[:],
                                  in1=psd[0:64, :], s0=self.bvhs[0:64],
                                  s1=2.0)
            cat = p["cat"].tile([128, 4 * Wh], F32, tag="cat")
            nc.scalar.activation(cat[64:128, :], psd[64:128, :], Act.Identity)
            uu = 2 * b + su
            nc.sync.dma_start(out=self.xloV[:, uu, :], in_=cat[64:128, :])
            nc.sync.dma_start(out=self.xhiV[:, uu, :], in_=xht[:])
        self.h1_prev, self.h1_prev_first = h1, first_row
        if self.dbg == "h1":
            nc.sync.dma_start(
                out=self.dbgp[:, b * 18 * Wp:(b + 1) * 18 * Wp],
                in_=h1f[:, 0:18 * Wp])
        elif self.dbg == "h2":
            nc.sync.dma_start(
                out=self.dbgp[:, b * 8 * W:(b + 1) * 8 * W], in_=h2f[:])


def _build(H, W, finalize=True, reps=1, dbg=None):
    return _Builder(H, W, finalize=finalize, reps=reps, dbg=dbg).build()


def _get_program(H, W, reps=1):
    key = (H, W, reps)
    if key not in _CACHE:
        _CACHE[key] = _build(H, W, reps=reps)
    return _CACHE[key]


def _prep_inputs(x, w1, b1, g1, be1, m1, v1, w2, b2, g2, be2, m2, v2,
                 wh, bh, gh, beh, mh, vh):
    x = np.asarray(x, dtype=np.float32)
    B, C, H, W = x.shape
    w1a, w1b, w1c, w2t, whca, bv1, bv2d, bvhd = _fold_params(
        np.asarray(w1, np.float32), np.asarray(b1, np.float32),
        np.asarray(g1, np.float32), np.asarray(be1, np.float32),
        np.asarray(m1, np.float32), np.asarray(v1, np.float32),
        np.asarray(w2, np.float32), np.asarray(b2, np.float32),
        np.asarray(g2, np.float32), np.asarray(be2, np.float32),
        np.asarray(m2, np.float32), np.asarray(v2, np.float32),
        np.asarray(wh, np.float32), np.asarray(bh, np.float32),
        np.asarray(gh, np.float32), np.asarray(beh, np.float32),
        np.asarray(mh, np.float32), np.asarray(vh, np.float32))
    import concourse.mybir as _mybir
    bf16 = _mybir.dt.np(_mybir.dt.bfloat16)
    xp = np.zeros((B, C, H + 5, W + 2), dtype=bf16)
    xp[:, :, 2:H + 2, 1:W + 1] = (0.5 * x).astype(bf16)
    in_maps = []
    for i in range(B):
        in_maps.append({
            "x": np.ascontiguousarray(xp[i].reshape(C, (H + 5) * (W + 2))),
            "w1a": w1a, "w1b": w1b, "w1c": w1c, "w2t": w2t, "whca": whca,
            "bv1": bv1, "bv2": bv2d, "bvh": bvhd,
        })
    return in_maps, B, C, H, W


def kernel(x, w1, b1, g1, be1, m1, v1, w2, b2, g2, be2, m2, v2,
           wh, bh, gh, beh, mh, vh):
    from concourse.bass_utils import run_bass_kernel_spmd

    in_maps, B, C, H, W = _prep_inputs(
        x, w1, b1, g1, be1, m1, v1, w2, b2, g2, be2, m2, v2,
        wh, bh, gh, beh, mh, vh)
    nc = _get_program(H, W)
    res = run_bass_kernel_spmd(nc, in_maps, list(range(B)), trace=False)
    if res.exec_time_ns is not None:
        print(f"HW exec time: {res.exec_time_ns} ns")
    H2, W2 = H // 2, W // 2
    x_low = np.stack([res.results[i]["x_low"].astype(np.float32)
                      .reshape(C, H2, W2) for i in range(B)])
    x_high = np.stack([res.results[i]["x_high"].astype(np.float32)
                       .reshape(C, H2, W2) for i in range(B)])
    return (x_low, x_high)
